# revision 15
# baseline (speedup 1.0000x reference)
"""Trainium2 Bass kernel for nn_Attention_49091476194121.

Single-head attention with a shared Q projection applied to q, k and v,
softmax (no scaling), then an output projection. Returns (out, weights)
exactly like the reference.

Sharding: 8 cores = 4 batches x 2 query-row halves. Each core handles one
(batch, s-half): it computes the full K/V projections for its batch
(duplicated across the pair) and its 1024 query rows end to end.

Precision: the q/k projections and the score matmul run in true fp32
(softmax exponentiates scores ~ +-130, so score errors are amplified);
the v projection, context matmul and output projection run in float32r
(TF32-class, ~1.5e-4 rel err, 4x faster on the PE).
"""

import os
import sys

sys.path.insert(0, "/opt/trn_rl_repo")

import ml_dtypes
import numpy as np

import concourse.bass as bass
import concourse.tile as tile
from concourse import bacc, mybir
from concourse.bass_utils import run_bass_kernel_spmd
from concourse.masks import make_identity

P = 128
D = 1024          # d_model
S_LOC = 1024      # query rows per core
T = 2048          # key/value rows per batch
DCH = D // P      # 8 chunks of the contraction/feature dims
SCH = S_LOC // P  # 8 query-row chunks
TCH = T // P      # 16 key-row chunks
NF = 512          # matmul free-dim tile

f32 = mybir.dt.float32
f32r = mybir.dt.float32r
bf16 = mybir.dt.bfloat16

# Set by kernel() when BASS_TRACE is enabled (see test.py).
LAST_EXEC_NS = None

_CACHE = {}


def _build_program():
    nc = bacc.Bacc("TRN2", target_bir_lowering=False, debug=False, num_devices=8)

    q_in = nc.dram_tensor("q_in", [S_LOC, D], f32, kind="ExternalInput").ap()
    k_in = nc.dram_tensor("k_in", [T // 2, D], f32, kind="ExternalInput").ap()
    v_in = nc.dram_tensor("v_in", [T // 2, D], f32, kind="ExternalInput").ap()
    Wq_h = nc.dram_tensor("Wq_h", [D, D], bf16, kind="ExternalInput").ap()
    Wq_l = nc.dram_tensor("Wq_l", [D, D], bf16, kind="ExternalInput").ap()
    Wq = nc.dram_tensor("Wq", [D, D], f32, kind="ExternalInput").ap()
    bq = nc.dram_tensor("bq", [D], f32, kind="ExternalInput").ap()
    Wo = nc.dram_tensor("Wo", [D, D], f32, kind="ExternalInput").ap()
    bo = nc.dram_tensor("bo", [D], f32, kind="ExternalInput").ap()

    w_out = nc.dram_tensor("w_out", [S_LOC, T], f32, kind="ExternalOutput").ap()
    o_out = nc.dram_tensor("o_out", [S_LOC, D], f32, kind="ExternalOutput").ap()

    # DRAM spills
    qpT_dh = nc.dram_tensor("qpT_dh", [D, S_LOC], bf16).ap()     # [e, s] hi
    qpT_dl = nc.dram_tensor("qpT_dl", [D, S_LOC], bf16).ap()     # [e, s] lo
    kloc_h = nc.dram_tensor("kloc_h", [D, T // 2], bf16).ap()    # local kpT half
    kloc_l = nc.dram_tensor("kloc_l", [D, T // 2], bf16).ap()
    kgat_h = nc.dram_tensor("kgat_h", [2 * D, T // 2], bf16).ap()
    kgat_l = nc.dram_tensor("kgat_l", [2 * D, T // 2], bf16).ap()
    vploc = nc.dram_tensor("vploc", [T // 2, D], f32r).ap()
    vp_d = nc.dram_tensor("vp_d", [T, D], f32r).ap()             # [t, e] gathered
    ctxT_d = nc.dram_tensor("ctxT_d", [D, S_LOC], f32r).ap()     # [e, s]
    wT_d = nc.dram_tensor("wT_d", [TCH, P, S_LOC], f32r).ap()    # [tc][t_in, s]

    with tile.TileContext(nc, pool_alloc_mode="queue") as tc:
        _emit(tc, nc, q_in, k_in, v_in, Wq_h, Wq_l, Wq, bq, Wo, bo,
              w_out, o_out, qpT_dh, qpT_dl, vp_d, wT_d,
              kloc_h, kloc_l, kgat_h, kgat_l, vploc, ctxT_d)

    nc.compile()
    return nc


def _transpose_rows_hl(nc, trpool, ident, row_ap, dst_h, dst_l, dst_col0, nch, scr):
    """PE-transpose + bf16 hi/lo split into dst_h/dst_l."""
    for dh in range((nch + 3) // 4):
        jmax = min(4, nch - dh * 4)
        tr = trpool.tile([P, 4, P], f32, tag="tr")
        for j in range(jmax):
            dch = dh * 4 + j
            nc.tensor.transpose(tr[:, j, :], row_ap[:, dch * P:(dch + 1) * P], ident[:])
        _split_hi_lo(
            nc, tr[:, :jmax, :],
            dst_h[:, dh * 4:dh * 4 + jmax, dst_col0:dst_col0 + P],
            dst_l[:, dh * 4:dh * 4 + jmax, dst_col0:dst_col0 + P],
            scr, [P, 4, P])


def _transpose_rows(nc, trpool, ident, row_ap, dst_st, dst_col0, nch):
    """PE-transpose row_ap ([128, nch*128], partition=rows) into dst_st
    ([128, nch, >=dst_col0+128], partition=cols), at free offset dst_col0."""
    for dh in range((nch + 3) // 4):
        jmax = min(4, nch - dh * 4)
        tr = trpool.tile([P, 4, P], f32, tag="tr")
        for j in range(jmax):
            dch = dh * 4 + j
            nc.tensor.transpose(tr[:, j, :], row_ap[:, dch * P:(dch + 1) * P], ident[:])
        nc.vector.tensor_copy(
            dst_st[:, dh * 4:dh * 4 + jmax, dst_col0:dst_col0 + P],
            tr[:, :jmax, :],
        )


def _split_hi_lo(nc, src_ap, hi_ap, lo_ap, scratch_pool, shape):
    """hi = bf16(src); lo = bf16(src - hi). src is f32 (SBUF or PSUM)."""
    nc.vector.tensor_copy(hi_ap, src_ap)
    h32 = scratch_pool.tile(shape, f32, tag="h32", name="h32")
    nc.vector.tensor_copy(h32[:], hi_ap)
    nc.vector.tensor_sub(lo_ap, src_ap, h32[:])


def _emit(tc, nc, q_in, k_in, v_in, Wq_hd, Wq_ld, Wq, bq, Wo, bo,
          w_out, o_out, qpT_dh, qpT_dl, vp_d, wT_d,
          kloc_h, kloc_l, kgat_h, kgat_l, vploc, ctxT_d):
    GROUPS = [[0, 1], [2, 3], [4, 5], [6, 7]]
    TQL = (T // 2) // NF   # 2 free-dim groups in the local t-half
    TCL = TCH // 2         # 8 row chunks in the local t-half

    ctx0 = tc.tile_pool(name="outer", bufs=1)
    with ctx0 as outer:
        ident = outer.tile([P, P], f32)
        make_identity(nc, ident[:])

        trp_cm = tc.tile_pool(name="trp", bufs=2, space="PSUM")
        with trp_cm as trpool:
            with (
                tc.tile_pool(name="wqbias", bufs=1) as wqp,
                tc.tile_pool(name="rows", bufs=2) as rows,
            ):
                bq_cols = wqp.tile([P, DCH], f32)    # bq_cols[p, ec] = bq[ec*128+p]
                nc.scalar.dma_start(bq_cols[:], bq.rearrange("(ec p) -> p ec", p=P))
                bq_row = wqp.tile([P, D], f32)       # bq broadcast to all partitions
                nc.scalar.dma_start(bq_row[:], bq.unsqueeze(0).to_broadcast((P, D)))
                wq_h = wqp.tile([P, DCH, D], bf16)   # Wq hi [d-part, dch, e]
                wq_l = wqp.tile([P, DCH, D], bf16)   # Wq lo
                for dch in range(DCH):
                    nc.scalar.dma_start(wq_h[:, dch, :], Wq_hd[dch * P:(dch + 1) * P, :])
                    nc.scalar.dma_start(wq_l[:, dch, :], Wq_ld[dch * P:(dch + 1) * P, :])

                # ---- K (local t-half): kpT_loc -> kloc_h/l -> AllGather ----
                with (
                    tc.tile_pool(name="scr", bufs=2) as scr,
                    tc.tile_pool(name="xst", bufs=2) as xst,
                    tc.tile_pool(name="spill", bufs=3) as spill,
                    tc.tile_pool(name="pp", bufs=2, space="PSUM") as pp,
                ):
                    for tq in range(TQL):
                        k_sh = xst.tile([P, DCH, NF], bf16, tag="xsth")
                        k_sl = xst.tile([P, DCH, NF], bf16, tag="xstl")
                        for ts in range(4):
                            r = rows.tile([P, D], f32, tag="row")
                            nc.sync.dma_start(r[:], k_in[(tq * 4 + ts) * P:(tq * 4 + ts + 1) * P, :])
                            _transpose_rows_hl(nc, trpool, ident, r[:], k_sh, k_sl, ts * P, DCH, scr)
                        for ecg in range(4):
                            ps = pp.tile([P, 2, NF], f32, tag="proj")
                            for d in range(DCH):
                                for e2 in range(2):
                                    ec = ecg * 2 + e2
                                    first = (d == 0)
                                    last = (d == DCH - 1)
                                    nc.tensor.matmul(
                                        ps[:, e2, :], wq_h[:, d, ec * P:(ec + 1) * P], k_sh[:, d, :],
                                        start=first, stop=False)
                                    nc.tensor.matmul(
                                        ps[:, e2, :], wq_h[:, d, ec * P:(ec + 1) * P], k_sl[:, d, :],
                                        start=False, stop=False)
                                    nc.tensor.matmul(
                                        ps[:, e2, :], wq_l[:, d, ec * P:(ec + 1) * P], k_sh[:, d, :],
                                        start=False, stop=last)
                            for e2 in range(2):
                                ec = ecg * 2 + e2
                                kf = scr.tile([P, NF], f32, tag="qf", name="kf")
                                nc.vector.tensor_scalar_add(kf[:], ps[:, e2, :], bq_cols[:, ec:ec + 1])
                                kh = spill.tile([P, NF], bf16, tag="qsph", name="kh")
                                kl = spill.tile([P, NF], bf16, tag="qspl", name="kl")
                                _split_hi_lo(nc, kf[:], kh[:], kl[:], scr, [P, NF])
                                nc.sync.dma_start(
                                    kloc_h[ec * P:(ec + 1) * P, tq * NF:(tq + 1) * NF], kh[:])
                                nc.sync.dma_start(
                                    kloc_l[ec * P:(ec + 1) * P, tq * NF:(tq + 1) * NF], kl[:])
                nc.gpsimd.collective_compute(
                    "AllGather", mybir.AluOpType.bypass, replica_groups=GROUPS,
                    ins=[kloc_h], outs=[kgat_h])
                nc.gpsimd.collective_compute(
                    "AllGather", mybir.AluOpType.bypass, replica_groups=GROUPS,
                    ins=[kloc_l], outs=[kgat_l])

                # ---- V (local t-half): vp_loc -> AllGather -> vp_d  (f32r) ----
                with (
                    tc.tile_pool(name="wqr", bufs=1) as wqr,
                    tc.tile_pool(name="vstp", bufs=2) as vstp,
                    tc.tile_pool(name="vspill", bufs=2) as vspill,
                    tc.tile_pool(name="ppv", bufs=2, space="PSUM") as ppv,
                ):
                    wq_r = wqr.tile([P, DCH, D], f32r)
                    for dch in range(DCH):
                        nc.vector.tensor_add(wq_r[:, dch, :], wq_h[:, dch, :], wq_l[:, dch, :])

                    for tch in range(TCL):
                        r = rows.tile([P, D], f32, tag="row")
                        nc.sync.dma_start(r[:], v_in[tch * P:(tch + 1) * P, :])
                        v_st = vstp.tile([P, DCH, P], f32r, tag="vst")
                        _transpose_rows(nc, trpool, ident, r[:], v_st, 0, DCH)
                        ps = ppv.tile([P, 2, NF], f32, tag="vproj")
                        for d in range(DCH):
                            for eh in range(2):
                                nc.tensor.matmul(
                                    ps[:, eh, :],
                                    v_st[:, d, :],
                                    wq_r[:, d, eh * NF:(eh + 1) * NF],
                                    start=(d == 0), stop=(d == DCH - 1),
                                )
                        vsb = vspill.tile([P, D], f32r, tag="vsp")
                        for eh in range(2):
                            nc.vector.tensor_add(
                                vsb[:, eh * NF:(eh + 1) * NF], ps[:, eh, :],
                                bq_row[:, eh * NF:(eh + 1) * NF],
                            )
                        nc.sync.dma_start(vploc[tch * P:(tch + 1) * P, :], vsb[:])
                nc.gpsimd.collective_compute(
                    "AllGather", mybir.AluOpType.bypass, replica_groups=GROUPS,
                    ins=[vploc], outs=[vp_d])

                # ---- Q: qpT[e, s] = (q_in @ Wq + bq)^T -> spill hi/lo ----
                with (
                    tc.tile_pool(name="scr3", bufs=2) as scr,
                    tc.tile_pool(name="xst3", bufs=2) as xst,
                    tc.tile_pool(name="spill3", bufs=3) as spill,
                    tc.tile_pool(name="pp3", bufs=2, space="PSUM") as pp,
                ):
                    for sh in range(2):
                        q_sh = xst.tile([P, DCH, NF], bf16, tag="xsth")
                        q_sl = xst.tile([P, DCH, NF], bf16, tag="xstl")
                        for ss in range(4):
                            r = rows.tile([P, D], f32, tag="row")
                            nc.scalar.dma_start(r[:], q_in[(sh * 4 + ss) * P:(sh * 4 + ss + 1) * P, :])
                            _transpose_rows_hl(nc, trpool, ident, r[:], q_sh, q_sl, ss * P, DCH, scr)
                        for ecg in range(4):
                            ps = pp.tile([P, 2, NF], f32, tag="proj")
                            for d in range(DCH):
                                for e2 in range(2):
                                    ec = ecg * 2 + e2
                                    first = (d == 0)
                                    last = (d == DCH - 1)
                                    nc.tensor.matmul(
                                        ps[:, e2, :], wq_h[:, d, ec * P:(ec + 1) * P], q_sh[:, d, :],
                                        start=first, stop=False)
                                    nc.tensor.matmul(
                                        ps[:, e2, :], wq_h[:, d, ec * P:(ec + 1) * P], q_sl[:, d, :],
                                        start=False, stop=False)
                                    nc.tensor.matmul(
                                        ps[:, e2, :], wq_l[:, d, ec * P:(ec + 1) * P], q_sh[:, d, :],
                                        start=False, stop=last)
                            for e2 in range(2):
                                ec = ecg * 2 + e2
                                qf = scr.tile([P, NF], f32, tag="qf", name="qf")
                                nc.vector.tensor_scalar_add(qf[:], ps[:, e2, :], bq_cols[:, ec:ec + 1])
                                sbh = spill.tile([P, NF], bf16, tag="qsph", name="sbh")
                                sbl = spill.tile([P, NF], bf16, tag="qspl", name="sbl")
                                _split_hi_lo(nc, qf[:], sbh[:], sbl[:], scr, [P, NF])
                                nc.sync.dma_start(
                                    qpT_dh[ec * P:(ec + 1) * P, sh * NF:(sh + 1) * NF], sbh[:])
                                nc.sync.dma_start(
                                    qpT_dl[ec * P:(ec + 1) * P, sh * NF:(sh + 1) * NF], sbl[:])

            # ---- S: scores/softmax/wT + fused-resident CTX ----
            with (
                tc.tile_pool(name="kres", bufs=1) as res,
                tc.tile_pool(name="qpt", bufs=1) as qptp,
                tc.tile_pool(name="vpt", bufs=1) as vptp,
            ):
                kpT_hs, kpT_ls = [], []
                for ec in range(DCH):
                    th = res.tile([P, T], bf16, tag=f"kpth{ec}", name=f"kpTh{ec}")
                    tl = res.tile([P, T], bf16, tag=f"kptl{ec}", name=f"kpTl{ec}")
                    for rk in range(2):
                        nc.scalar.dma_start(
                            th[:, rk * (T // 2):(rk + 1) * (T // 2)],
                            kgat_h[rk * D + ec * P:rk * D + (ec + 1) * P, :])
                        nc.scalar.dma_start(
                            tl[:, rk * (T // 2):(rk + 1) * (T // 2)],
                            kgat_l[rk * D + ec * P:rk * D + (ec + 1) * P, :])
                    kpT_hs.append(th)
                    kpT_ls.append(tl)
                qpT_hs, qpT_ls = [], []
                for ec in range(DCH):
                    th = qptp.tile([P, S_LOC], bf16, tag=f"qpth{ec}", name=f"qpTh{ec}")
                    nc.scalar.dma_start(th[:], qpT_dh[ec * P:(ec + 1) * P, :])
                    tl = qptp.tile([P, S_LOC], bf16, tag=f"qptl{ec}", name=f"qpTl{ec}")
                    nc.scalar.dma_start(tl[:], qpT_dl[ec * P:(ec + 1) * P, :])
                    qpT_hs.append(th)
                    qpT_ls.append(tl)
                vp_ts = []
                for tch in range(TCH):
                    t = vptp.tile([P, D], f32r, tag=f"vp{tch}", name=f"vpt{tch}")
                    nc.scalar.dma_start(t[:], vp_d[tch * P:(tch + 1) * P, :])
                    vp_ts.append(t)

                with (
                    tc.tile_pool(name="sm", bufs=2) as sm,
                    tc.tile_pool(name="scp", bufs=5, space="PSUM") as scp,
                ):
                    for sc in range(SCH):
                        sct = [scp.tile([P, NF], f32, tag="sc", name=f"sct{i}") for i in range(4)]
                        for ec in range(DCH):
                            first = (ec == 0)
                            last = (ec == DCH - 1)
                            for tq in range(4):
                                nc.tensor.matmul(
                                    sct[tq][:],
                                    qpT_hs[ec][:, sc * P:(sc + 1) * P],
                                    kpT_hs[ec][:, tq * NF:(tq + 1) * NF],
                                    start=first, stop=False)
                            for tq in range(4):
                                nc.tensor.matmul(
                                    sct[tq][:],
                                    qpT_hs[ec][:, sc * P:(sc + 1) * P],
                                    kpT_ls[ec][:, tq * NF:(tq + 1) * NF],
                                    start=False, stop=False)
                            for tq in range(4):
                                nc.tensor.matmul(
                                    sct[tq][:],
                                    qpT_ls[ec][:, sc * P:(sc + 1) * P],
                                    kpT_hs[ec][:, tq * NF:(tq + 1) * NF],
                                    start=False, stop=last)
                        sc_sb = sm.tile([P, T], f32, tag="scsb")
                        for tq in range(4):
                            nc.vector.tensor_copy(sc_sb[:, tq * NF:(tq + 1) * NF], sct[tq][:])
                        pmax = sm.tile([P, 4], f32, tag="pmax")
                        for tq in range(4):
                            nc.vector.tensor_reduce(
                                pmax[:, tq:tq + 1], sc_sb[:, tq * NF:(tq + 1) * NF],
                                axis=mybir.AxisListType.X, op=mybir.AluOpType.max,
                            )
                        negmax = sm.tile([P, 1], f32, tag="negmax")
                        nc.vector.tensor_reduce(
                            negmax[:], pmax[:], axis=mybir.AxisListType.X,
                            op=mybir.AluOpType.max, negate=True,
                        )
                        w_sb = sm.tile([P, T], f32, tag="wsb", bufs=1)
                        sums = sm.tile([P, 4], f32, tag="sums")
                        for tq in range(4):
                            nc.scalar.activation(
                                w_sb[:, tq * NF:(tq + 1) * NF], sc_sb[:, tq * NF:(tq + 1) * NF],
                                mybir.ActivationFunctionType.Exp,
                                bias=negmax[:], scale=1.0,
                                accum_out=sums[:, tq:tq + 1],
                            )
                        stot = sm.tile([P, 1], f32, tag="stot")
                        nc.vector.tensor_reduce(
                            stot[:], sums[:], axis=mybir.AxisListType.X, op=mybir.AluOpType.add,
                        )
                        recip = sm.tile([P, 1], f32, tag="recip")
                        nc.vector.reciprocal(recip[:], stot[:])
                        for tq in range(4):
                            nc.vector.tensor_scalar_mul(
                                w_sb[:, tq * NF:(tq + 1) * NF],
                                w_sb[:, tq * NF:(tq + 1) * NF], recip[:],
                            )
                        nc.sync.dma_start(w_out[sc * P:(sc + 1) * P, :], w_sb[:])

                        wT_sb = sm.tile([P, TCH, P], f32r, tag="wtsb")
                        for th in range(4):
                            tr = trpool.tile([P, 4, P], f32, tag="tr")
                            for j in range(4):
                                tch = th * 4 + j
                                nc.tensor.transpose(
                                    tr[:, j, :], w_sb[:, tch * P:(tch + 1) * P], ident[:]
                                )
                            nc.vector.tensor_copy(wT_sb[:, th * 4:(th + 1) * 4, :], tr[:])
                        nc.sync.dma_start(
                            wT_d.rearrange("tc p s -> p tc s")[:, :, sc * P:(sc + 1) * P],
                            wT_sb[:],
                        )

                # ---- context: ctxT[e, s] = vp^T @ w^T  (f32r) -> ctxT_d ----
                with (
                    tc.tile_pool(name="wtin", bufs=3) as wtin,
                    tc.tile_pool(name="cxs", bufs=2) as cxs,
                    tc.tile_pool(name="cxp", bufs=1, space="PSUM") as cxp,
                ):
                    for sh in range(2):
                        for g in range(2):
                            ps = cxp.tile([P, 4, NF], f32, tag="cx", name=f"cx{sh}{g}")
                            for tch in range(TCH):
                                wt = wtin.tile([P, NF], f32r, tag="wt")
                                nc.scalar.dma_start(
                                    wt[:], wT_d[tch, :, sh * NF:(sh + 1) * NF]
                                )
                                for e4 in range(4):
                                    ec = g * 4 + e4
                                    nc.tensor.matmul(
                                        ps[:, e4, :],
                                        vp_ts[tch][:, ec * P:(ec + 1) * P],
                                        wt[:],
                                        start=(tch == 0), stop=(tch == TCH - 1),
                                    )
                            st = cxs.tile([P, 4, NF], f32r, tag="cxs", name=f"cxs{sh}{g}")
                            nc.vector.tensor_copy(st[:], ps[:])
                            nc.sync.dma_start(
                                ctxT_d.rearrange("(a p) s -> p a s", p=P)[
                                    :, g * 4:(g + 1) * 4, sh * NF:(sh + 1) * NF],
                                st[:],
                            )

        # ---- output projection: out[s, f] = ctxT^T @ Wo + bo  (f32r) ----
        with (
            tc.tile_pool(name="wo", bufs=1) as wop,
            tc.tile_pool(name="osb", bufs=2) as osb,
            tc.tile_pool(name="oxp", bufs=2, space="PSUM") as oxp,
        ):
            wo_ts = []
            for ec in range(DCH):
                raw = wop.tile([P, D], f32, tag="woraw", name=f"woraw{ec}", bufs=3)
                nc.scalar.dma_start(raw[:], Wo[ec * P:(ec + 1) * P, :])
                t = wop.tile([P, D], f32r, tag=f"wo{ec}", name=f"wo{ec}")
                nc.vector.tensor_copy(t[:], raw[:])
                wo_ts.append(t)
            bo_row = wop.tile([P, D], f32)
            nc.scalar.dma_start(bo_row[:], bo.unsqueeze(0).to_broadcast((P, D)))
            ctx_ts = []
            for ec in range(DCH):
                t = wop.tile([P, S_LOC], f32r, tag=f"ctx{ec}", name=f"ctxt{ec}")
                nc.scalar.dma_start(t[:], ctxT_d[ec * P:(ec + 1) * P, :])
                ctx_ts.append(t)

            for sc in range(SCH):
                ps = oxp.tile([P, 2, NF], f32, tag="ox")
                for ec in range(DCH):
                    for fh in range(2):
                        nc.tensor.matmul(
                            ps[:, fh, :],
                            ctx_ts[ec][:, sc * P:(sc + 1) * P],
                            wo_ts[ec][:, fh * NF:(fh + 1) * NF],
                            start=(ec == 0), stop=(ec == DCH - 1),
                        )
                ob = osb.tile([P, D], f32, tag="ob")
                for fh in range(2):
                    nc.vector.tensor_add(
                        ob[:, fh * NF:(fh + 1) * NF], ps[:, fh, :],
                        bo_row[:, fh * NF:(fh + 1) * NF],
                    )
                nc.sync.dma_start(o_out[sc * P:(sc + 1) * P, :], ob[:])


def _get_program():
    if "nc" not in _CACHE:
        _CACHE["nc"] = _build_program()
    return _CACHE["nc"]


def kernel(query, key, value, Wq, bq, Wo, bo):
    global LAST_EXEC_NS
    query = np.ascontiguousarray(np.asarray(query, dtype=np.float32))
    key = np.ascontiguousarray(np.asarray(key, dtype=np.float32))
    value = np.ascontiguousarray(np.asarray(value, dtype=np.float32))
    Wq = np.ascontiguousarray(np.asarray(Wq, dtype=np.float32))
    bq = np.ascontiguousarray(np.asarray(bq, dtype=np.float32))
    Wo = np.ascontiguousarray(np.asarray(Wo, dtype=np.float32))
    bo = np.ascontiguousarray(np.asarray(bo, dtype=np.float32))

    B, S, Dm = query.shape
    assert (B, S, Dm) == (4, 2048, 1024), (B, S, Dm)

    nc = _get_program()
    wq_h = Wq.astype(ml_dtypes.bfloat16)
    wq_l = (Wq - wq_h.astype(np.float32)).astype(ml_dtypes.bfloat16)
    in_maps = []
    for c in range(8):
        b, sh = c // 2, c % 2
        in_maps.append({
            "q_in": np.ascontiguousarray(query[b, sh * S_LOC:(sh + 1) * S_LOC]),
            "k_in": np.ascontiguousarray(key[b, sh * (T // 2):(sh + 1) * (T // 2)]),
            "v_in": np.ascontiguousarray(value[b, sh * (T // 2):(sh + 1) * (T // 2)]),
            "Wq": Wq, "Wq_h": wq_h, "Wq_l": wq_l,
            "bq": bq, "Wo": Wo, "bo": bo,
        })

    res = run_bass_kernel_spmd(nc, in_maps, core_ids=list(range(8)))
    LAST_EXEC_NS = res.exec_time_ns

    out = np.empty((B, S, Dm), dtype=np.float32)
    weights = np.empty((B, S, T), dtype=np.float32)
    for c in range(8):
        b, sh = c // 2, c % 2
        out[b, sh * S_LOC:(sh + 1) * S_LOC] = res.results[c]["o_out"]
        weights[b, sh * S_LOC:(sh + 1) * S_LOC] = res.results[c]["w_out"]
    return out, weights


# revision 16
# speedup vs baseline: 1.0142x; 1.0142x over previous
"""Trainium2 Bass kernel for nn_Attention_49091476194121.

Single-head attention with a shared Q projection applied to q, k and v,
softmax (no scaling), then an output projection. Returns (out, weights)
exactly like the reference.

Sharding: 8 cores = 4 batches x 2 query-row halves. Each core handles one
(batch, s-half): it computes the full K/V projections for its batch
(duplicated across the pair) and its 1024 query rows end to end.

Precision: the q/k projections and the score matmul run in true fp32
(softmax exponentiates scores ~ +-130, so score errors are amplified);
the v projection, context matmul and output projection run in float32r
(TF32-class, ~1.5e-4 rel err, 4x faster on the PE).
"""

import os
import sys

sys.path.insert(0, "/opt/trn_rl_repo")

import ml_dtypes
import numpy as np

import concourse.bass as bass
import concourse.tile as tile
from concourse import bacc, mybir
from concourse.bass_utils import run_bass_kernel_spmd
from concourse.masks import make_identity

P = 128
D = 1024          # d_model
S_LOC = 1024      # query rows per core
T = 2048          # key/value rows per batch
DCH = D // P      # 8 chunks of the contraction/feature dims
SCH = S_LOC // P  # 8 query-row chunks
TCH = T // P      # 16 key-row chunks
NF = 512          # matmul free-dim tile

f32 = mybir.dt.float32
f32r = mybir.dt.float32r
bf16 = mybir.dt.bfloat16

# Set by kernel() when BASS_TRACE is enabled (see test.py).
LAST_EXEC_NS = None

_CACHE = {}


def _build_program():
    nc = bacc.Bacc("TRN2", target_bir_lowering=False, debug=False, num_devices=8)

    q_in = nc.dram_tensor("q_in", [S_LOC, D], f32, kind="ExternalInput").ap()
    k_in = nc.dram_tensor("k_in", [T // 2, D], f32, kind="ExternalInput").ap()
    v_in = nc.dram_tensor("v_in", [T // 2, D], f32, kind="ExternalInput").ap()
    Wq_h = nc.dram_tensor("Wq_h", [D, D], bf16, kind="ExternalInput").ap()
    Wq_l = nc.dram_tensor("Wq_l", [D, D], bf16, kind="ExternalInput").ap()
    Wq = nc.dram_tensor("Wq", [D, D], f32, kind="ExternalInput").ap()
    bq = nc.dram_tensor("bq", [D], f32, kind="ExternalInput").ap()
    Wo = nc.dram_tensor("Wo", [D, D], f32, kind="ExternalInput").ap()
    bo = nc.dram_tensor("bo", [D], f32, kind="ExternalInput").ap()

    w_out = nc.dram_tensor("w_out", [S_LOC, T], f32, kind="ExternalOutput").ap()
    o_out = nc.dram_tensor("o_out", [S_LOC, D], f32, kind="ExternalOutput").ap()

    # DRAM spills
    qpT_dh = nc.dram_tensor("qpT_dh", [D, S_LOC], bf16).ap()     # [e, s] hi
    qpT_dl = nc.dram_tensor("qpT_dl", [D, S_LOC], bf16).ap()     # [e, s] lo
    kloc_h = nc.dram_tensor("kloc_h", [D, T // 2], bf16).ap()    # local kpT half
    kloc_l = nc.dram_tensor("kloc_l", [D, T // 2], bf16).ap()
    kgat_h = nc.dram_tensor("kgat_h", [2 * D, T // 2], bf16).ap()
    kgat_l = nc.dram_tensor("kgat_l", [2 * D, T // 2], bf16).ap()
    vploc = nc.dram_tensor("vploc", [T // 2, D], f32r).ap()
    vp_d = nc.dram_tensor("vp_d", [T, D], f32r).ap()             # [t, e] gathered
    ctxT_d = nc.dram_tensor("ctxT_d", [D, S_LOC], f32r).ap()     # [e, s]
    wT_d = nc.dram_tensor("wT_d", [TCH, P, S_LOC], f32r).ap()    # [tc][t_in, s]

    with tile.TileContext(nc, pool_alloc_mode="queue") as tc:
        _emit(tc, nc, q_in, k_in, v_in, Wq_h, Wq_l, Wq, bq, Wo, bo,
              w_out, o_out, qpT_dh, qpT_dl, vp_d, wT_d,
              kloc_h, kloc_l, kgat_h, kgat_l, vploc, ctxT_d)

    nc.compile()
    return nc


def _transpose_rows_hl(nc, trpool, ident, row_ap, dst_h, dst_l, dst_col0, nch, scr):
    """PE-transpose + bf16 hi/lo split into dst_h/dst_l."""
    for dh in range((nch + 3) // 4):
        jmax = min(4, nch - dh * 4)
        tr = trpool.tile([P, 4, P], f32, tag="tr")
        for j in range(jmax):
            dch = dh * 4 + j
            nc.tensor.transpose(tr[:, j, :], row_ap[:, dch * P:(dch + 1) * P], ident[:])
        _split_hi_lo(
            nc, tr[:, :jmax, :],
            dst_h[:, dh * 4:dh * 4 + jmax, dst_col0:dst_col0 + P],
            dst_l[:, dh * 4:dh * 4 + jmax, dst_col0:dst_col0 + P],
            scr, [P, 4, P])


def _transpose_rows(nc, trpool, ident, row_ap, dst_st, dst_col0, nch):
    """PE-transpose row_ap ([128, nch*128], partition=rows) into dst_st
    ([128, nch, >=dst_col0+128], partition=cols), at free offset dst_col0."""
    for dh in range((nch + 3) // 4):
        jmax = min(4, nch - dh * 4)
        tr = trpool.tile([P, 4, P], f32, tag="tr")
        for j in range(jmax):
            dch = dh * 4 + j
            nc.tensor.transpose(tr[:, j, :], row_ap[:, dch * P:(dch + 1) * P], ident[:])
        nc.vector.tensor_copy(
            dst_st[:, dh * 4:dh * 4 + jmax, dst_col0:dst_col0 + P],
            tr[:, :jmax, :],
        )


def _split_hi_lo(nc, src_ap, hi_ap, lo_ap, scratch_pool, shape):
    """hi = bf16(src); lo = bf16(src - hi). src is f32 (SBUF or PSUM)."""
    nc.vector.tensor_copy(hi_ap, src_ap)
    h32 = scratch_pool.tile(shape, f32, tag="h32", name="h32")
    nc.vector.tensor_copy(h32[:], hi_ap)
    nc.vector.tensor_sub(lo_ap, src_ap, h32[:])


def _emit(tc, nc, q_in, k_in, v_in, Wq_hd, Wq_ld, Wq, bq, Wo, bo,
          w_out, o_out, qpT_dh, qpT_dl, vp_d, wT_d,
          kloc_h, kloc_l, kgat_h, kgat_l, vploc, ctxT_d):
    GROUPS = [[0, 1], [2, 3], [4, 5], [6, 7]]
    TQL = (T // 2) // NF   # 2 free-dim groups in the local t-half
    TCL = TCH // 2         # 8 row chunks in the local t-half

    ctx0 = tc.tile_pool(name="outer", bufs=1)
    with ctx0 as outer:
        ident = outer.tile([P, P], f32)
        make_identity(nc, ident[:])

        trp_cm = tc.tile_pool(name="trp", bufs=2, space="PSUM")
        with trp_cm as trpool:
            with (
                tc.tile_pool(name="wqbias", bufs=1) as wqp,
                tc.tile_pool(name="rows", bufs=2) as rows,
            ):
                bq_cols = wqp.tile([P, DCH], f32)    # bq_cols[p, ec] = bq[ec*128+p]
                nc.scalar.dma_start(bq_cols[:], bq.rearrange("(ec p) -> p ec", p=P))
                bq_row = wqp.tile([P, D], f32)       # bq broadcast to all partitions
                nc.scalar.dma_start(bq_row[:], bq.unsqueeze(0).to_broadcast((P, D)))
                wq_h = wqp.tile([P, DCH, D], bf16)   # Wq hi [d-part, dch, e]
                wq_l = wqp.tile([P, DCH, D], bf16)   # Wq lo
                for dch in range(DCH):
                    nc.scalar.dma_start(wq_h[:, dch, :], Wq_hd[dch * P:(dch + 1) * P, :])
                    nc.scalar.dma_start(wq_l[:, dch, :], Wq_ld[dch * P:(dch + 1) * P, :])

                # ---- K (local t-half): kpT_loc -> kloc_h/l -> AllGather ----
                with (
                    tc.tile_pool(name="scr", bufs=2) as scr,
                    tc.tile_pool(name="xst", bufs=2) as xst,
                    tc.tile_pool(name="spill", bufs=3) as spill,
                    tc.tile_pool(name="pp", bufs=2, space="PSUM") as pp,
                ):
                    for tq in range(TQL):
                        k_sh = xst.tile([P, DCH, NF], bf16, tag="xsth")
                        k_sl = xst.tile([P, DCH, NF], bf16, tag="xstl")
                        for ts in range(4):
                            r = rows.tile([P, D], f32, tag="row")
                            nc.sync.dma_start(r[:], k_in[(tq * 4 + ts) * P:(tq * 4 + ts + 1) * P, :])
                            _transpose_rows_hl(nc, trpool, ident, r[:], k_sh, k_sl, ts * P, DCH, scr)
                        for ecg in range(4):
                            ps = pp.tile([P, 2, NF], f32, tag="proj")
                            for d in range(DCH):
                                for e2 in range(2):
                                    ec = ecg * 2 + e2
                                    first = (d == 0)
                                    last = (d == DCH - 1)
                                    nc.tensor.matmul(
                                        ps[:, e2, :], wq_h[:, d, ec * P:(ec + 1) * P], k_sh[:, d, :],
                                        start=first, stop=False)
                                    nc.tensor.matmul(
                                        ps[:, e2, :], wq_h[:, d, ec * P:(ec + 1) * P], k_sl[:, d, :],
                                        start=False, stop=False)
                                    nc.tensor.matmul(
                                        ps[:, e2, :], wq_l[:, d, ec * P:(ec + 1) * P], k_sh[:, d, :],
                                        start=False, stop=last)
                            for e2 in range(2):
                                ec = ecg * 2 + e2
                                kf = scr.tile([P, NF], f32, tag="qf", name="kf")
                                nc.vector.tensor_scalar_add(kf[:], ps[:, e2, :], bq_cols[:, ec:ec + 1])
                                kh = spill.tile([P, NF], bf16, tag="qsph", name="kh")
                                kl = spill.tile([P, NF], bf16, tag="qspl", name="kl")
                                _split_hi_lo(nc, kf[:], kh[:], kl[:], scr, [P, NF])
                                nc.sync.dma_start(
                                    kloc_h[ec * P:(ec + 1) * P, tq * NF:(tq + 1) * NF], kh[:])
                                nc.sync.dma_start(
                                    kloc_l[ec * P:(ec + 1) * P, tq * NF:(tq + 1) * NF], kl[:])
                nc.gpsimd.collective_compute(
                    "AllGather", mybir.AluOpType.bypass, replica_groups=GROUPS,
                    ins=[kloc_h], outs=[kgat_h])
                nc.gpsimd.collective_compute(
                    "AllGather", mybir.AluOpType.bypass, replica_groups=GROUPS,
                    ins=[kloc_l], outs=[kgat_l])

                # ---- V (local t-half): vp_loc -> AllGather -> vp_d  (f32r) ----
                with (
                    tc.tile_pool(name="wqr", bufs=1) as wqr,
                    tc.tile_pool(name="vstp", bufs=2) as vstp,
                    tc.tile_pool(name="vspill", bufs=2) as vspill,
                    tc.tile_pool(name="ppv", bufs=2, space="PSUM") as ppv,
                ):
                    wq_r = wqr.tile([P, DCH, D], f32r)
                    for dch in range(DCH):
                        nc.vector.tensor_add(wq_r[:, dch, :], wq_h[:, dch, :], wq_l[:, dch, :])

                    for tch in range(TCL):
                        r = rows.tile([P, D], f32, tag="row")
                        nc.sync.dma_start(r[:], v_in[tch * P:(tch + 1) * P, :])
                        v_st = vstp.tile([P, DCH, P], f32r, tag="vst")
                        _transpose_rows(nc, trpool, ident, r[:], v_st, 0, DCH)
                        ps = ppv.tile([P, 2, NF], f32, tag="vproj")
                        for d in range(DCH):
                            for eh in range(2):
                                nc.tensor.matmul(
                                    ps[:, eh, :],
                                    v_st[:, d, :],
                                    wq_r[:, d, eh * NF:(eh + 1) * NF],
                                    start=(d == 0), stop=(d == DCH - 1),
                                )
                        vsb = vspill.tile([P, D], f32r, tag="vsp")
                        for eh in range(2):
                            nc.vector.tensor_add(
                                vsb[:, eh * NF:(eh + 1) * NF], ps[:, eh, :],
                                bq_row[:, eh * NF:(eh + 1) * NF],
                            )
                        nc.sync.dma_start(vploc[tch * P:(tch + 1) * P, :], vsb[:])
                nc.gpsimd.collective_compute(
                    "AllGather", mybir.AluOpType.bypass, replica_groups=GROUPS,
                    ins=[vploc], outs=[vp_d])

                # ---- Q: qpT[e, s] = (q_in @ Wq + bq)^T -> spill hi/lo ----
                with (
                    tc.tile_pool(name="scr3", bufs=2) as scr,
                    tc.tile_pool(name="xst3", bufs=2) as xst,
                    tc.tile_pool(name="spill3", bufs=3) as spill,
                    tc.tile_pool(name="pp3", bufs=2, space="PSUM") as pp,
                ):
                    for sh in range(2):
                        q_sh = xst.tile([P, DCH, NF], bf16, tag="xsth")
                        q_sl = xst.tile([P, DCH, NF], bf16, tag="xstl")
                        for ss in range(4):
                            r = rows.tile([P, D], f32, tag="row")
                            nc.scalar.dma_start(r[:], q_in[(sh * 4 + ss) * P:(sh * 4 + ss + 1) * P, :])
                            _transpose_rows_hl(nc, trpool, ident, r[:], q_sh, q_sl, ss * P, DCH, scr)
                        for ecg in range(4):
                            ps = pp.tile([P, 2, NF], f32, tag="proj")
                            for d in range(DCH):
                                for e2 in range(2):
                                    ec = ecg * 2 + e2
                                    first = (d == 0)
                                    last = (d == DCH - 1)
                                    nc.tensor.matmul(
                                        ps[:, e2, :], wq_h[:, d, ec * P:(ec + 1) * P], q_sh[:, d, :],
                                        start=first, stop=False)
                                    nc.tensor.matmul(
                                        ps[:, e2, :], wq_h[:, d, ec * P:(ec + 1) * P], q_sl[:, d, :],
                                        start=False, stop=False)
                                    nc.tensor.matmul(
                                        ps[:, e2, :], wq_l[:, d, ec * P:(ec + 1) * P], q_sh[:, d, :],
                                        start=False, stop=last)
                            for e2 in range(2):
                                ec = ecg * 2 + e2
                                qf = scr.tile([P, NF], f32, tag="qf", name="qf")
                                nc.vector.tensor_scalar_add(qf[:], ps[:, e2, :], bq_cols[:, ec:ec + 1])
                                sbh = spill.tile([P, NF], bf16, tag="qsph", name="sbh")
                                sbl = spill.tile([P, NF], bf16, tag="qspl", name="sbl")
                                _split_hi_lo(nc, qf[:], sbh[:], sbl[:], scr, [P, NF])
                                nc.sync.dma_start(
                                    qpT_dh[ec * P:(ec + 1) * P, sh * NF:(sh + 1) * NF], sbh[:])
                                nc.sync.dma_start(
                                    qpT_dl[ec * P:(ec + 1) * P, sh * NF:(sh + 1) * NF], sbl[:])

            # ---- S: scores/softmax/wT + fused-resident CTX ----
            with (
                tc.tile_pool(name="kres", bufs=1) as res,
                tc.tile_pool(name="qpt", bufs=1) as qptp,
                tc.tile_pool(name="vpt", bufs=1) as vptp,
            ):
                kpT_hs, kpT_ls = [], []
                for ec in range(DCH):
                    th = res.tile([P, T], bf16, tag=f"kpth{ec}", name=f"kpTh{ec}")
                    tl = res.tile([P, T], bf16, tag=f"kptl{ec}", name=f"kpTl{ec}")
                    for rk in range(2):
                        nc.scalar.dma_start(
                            th[:, rk * (T // 2):(rk + 1) * (T // 2)],
                            kgat_h[rk * D + ec * P:rk * D + (ec + 1) * P, :])
                        nc.scalar.dma_start(
                            tl[:, rk * (T // 2):(rk + 1) * (T // 2)],
                            kgat_l[rk * D + ec * P:rk * D + (ec + 1) * P, :])
                    kpT_hs.append(th)
                    kpT_ls.append(tl)
                qpT_hs, qpT_ls = [], []
                for ec in range(DCH):
                    th = qptp.tile([P, S_LOC], bf16, tag=f"qpth{ec}", name=f"qpTh{ec}")
                    nc.scalar.dma_start(th[:], qpT_dh[ec * P:(ec + 1) * P, :])
                    tl = qptp.tile([P, S_LOC], bf16, tag=f"qptl{ec}", name=f"qpTl{ec}")
                    nc.scalar.dma_start(tl[:], qpT_dl[ec * P:(ec + 1) * P, :])
                    qpT_hs.append(th)
                    qpT_ls.append(tl)
                vp_ts = []
                for tch in range(TCH):
                    t = vptp.tile([P, D], f32r, tag=f"vp{tch}", name=f"vpt{tch}")
                    nc.scalar.dma_start(t[:], vp_d[tch * P:(tch + 1) * P, :])
                    vp_ts.append(t)

                with (
                    tc.tile_pool(name="sm", bufs=2) as sm,
                    tc.tile_pool(name="scp", bufs=6, space="PSUM") as scp,
                ):
                    for sc in range(SCH):
                        sct = [scp.tile([P, NF], f32, tag="sc", name=f"sct{i}") for i in range(4)]
                        for ec in range(DCH):
                            first = (ec == 0)
                            last = (ec == DCH - 1)
                            for tq in range(4):
                                nc.tensor.matmul(
                                    sct[tq][:],
                                    qpT_hs[ec][:, sc * P:(sc + 1) * P],
                                    kpT_hs[ec][:, tq * NF:(tq + 1) * NF],
                                    start=first, stop=False)
                            for tq in range(4):
                                nc.tensor.matmul(
                                    sct[tq][:],
                                    qpT_hs[ec][:, sc * P:(sc + 1) * P],
                                    kpT_ls[ec][:, tq * NF:(tq + 1) * NF],
                                    start=False, stop=False)
                            for tq in range(4):
                                nc.tensor.matmul(
                                    sct[tq][:],
                                    qpT_ls[ec][:, sc * P:(sc + 1) * P],
                                    kpT_hs[ec][:, tq * NF:(tq + 1) * NF],
                                    start=False, stop=last)
                        sc_sb = sm.tile([P, T], f32, tag="scsb")
                        for tq in range(4):
                            nc.vector.tensor_copy(sc_sb[:, tq * NF:(tq + 1) * NF], sct[tq][:])
                        pmax = sm.tile([P, 4], f32, tag="pmax")
                        for tq in range(4):
                            nc.vector.tensor_reduce(
                                pmax[:, tq:tq + 1], sc_sb[:, tq * NF:(tq + 1) * NF],
                                axis=mybir.AxisListType.X, op=mybir.AluOpType.max,
                            )
                        negmax = sm.tile([P, 1], f32, tag="negmax")
                        nc.vector.tensor_reduce(
                            negmax[:], pmax[:], axis=mybir.AxisListType.X,
                            op=mybir.AluOpType.max, negate=True,
                        )
                        w_sb = sm.tile([P, T], f32, tag="wsb", bufs=1)
                        sums = sm.tile([P, 4], f32, tag="sums")
                        for tq in range(4):
                            nc.scalar.activation(
                                w_sb[:, tq * NF:(tq + 1) * NF], sc_sb[:, tq * NF:(tq + 1) * NF],
                                mybir.ActivationFunctionType.Exp,
                                bias=negmax[:], scale=1.0,
                                accum_out=sums[:, tq:tq + 1],
                            )
                        stot = sm.tile([P, 1], f32, tag="stot")
                        nc.vector.tensor_reduce(
                            stot[:], sums[:], axis=mybir.AxisListType.X, op=mybir.AluOpType.add,
                        )
                        recip = sm.tile([P, 1], f32, tag="recip")
                        nc.vector.reciprocal(recip[:], stot[:])
                        for tq in range(4):
                            nc.vector.tensor_scalar_mul(
                                w_sb[:, tq * NF:(tq + 1) * NF],
                                w_sb[:, tq * NF:(tq + 1) * NF], recip[:],
                            )
                        nc.sync.dma_start(w_out[sc * P:(sc + 1) * P, :], w_sb[:])

                        wT_sb = sm.tile([P, TCH, P], f32r, tag="wtsb")
                        for th in range(4):
                            tr = trpool.tile([P, 4, P], f32, tag="tr")
                            for j in range(4):
                                tch = th * 4 + j
                                nc.tensor.transpose(
                                    tr[:, j, :], w_sb[:, tch * P:(tch + 1) * P], ident[:]
                                )
                            nc.vector.tensor_copy(wT_sb[:, th * 4:(th + 1) * 4, :], tr[:])
                        nc.sync.dma_start(
                            wT_d.rearrange("tc p s -> p tc s")[:, :, sc * P:(sc + 1) * P],
                            wT_sb[:],
                        )

                # ---- context: ctxT[e, s] = vp^T @ w^T  (f32r) -> ctxT_d ----
                with (
                    tc.tile_pool(name="wtin", bufs=4) as wtin,
                    tc.tile_pool(name="cxs", bufs=2) as cxs,
                    tc.tile_pool(name="cxp", bufs=1, space="PSUM") as cxp,
                ):
                    for sh in range(2):
                        for g in range(2):
                            ps = cxp.tile([P, 4, NF], f32, tag="cx", name=f"cx{sh}{g}")
                            for tch in range(TCH):
                                wt = wtin.tile([P, NF], f32r, tag="wt")
                                nc.scalar.dma_start(
                                    wt[:], wT_d[tch, :, sh * NF:(sh + 1) * NF]
                                )
                                for e4 in range(4):
                                    ec = g * 4 + e4
                                    nc.tensor.matmul(
                                        ps[:, e4, :],
                                        vp_ts[tch][:, ec * P:(ec + 1) * P],
                                        wt[:],
                                        start=(tch == 0), stop=(tch == TCH - 1),
                                    )
                            st = cxs.tile([P, 4, NF], f32r, tag="cxs", name=f"cxs{sh}{g}")
                            nc.vector.tensor_copy(st[:], ps[:])
                            nc.sync.dma_start(
                                ctxT_d.rearrange("(a p) s -> p a s", p=P)[
                                    :, g * 4:(g + 1) * 4, sh * NF:(sh + 1) * NF],
                                st[:],
                            )

        # ---- output projection: out[s, f] = ctxT^T @ Wo + bo  (f32r) ----
        with (
            tc.tile_pool(name="wo", bufs=1) as wop,
            tc.tile_pool(name="osb", bufs=2) as osb,
            tc.tile_pool(name="oxp", bufs=2, space="PSUM") as oxp,
        ):
            wo_ts = []
            for ec in range(DCH):
                raw = wop.tile([P, D], f32, tag="woraw", name=f"woraw{ec}", bufs=3)
                nc.scalar.dma_start(raw[:], Wo[ec * P:(ec + 1) * P, :])
                t = wop.tile([P, D], f32r, tag=f"wo{ec}", name=f"wo{ec}")
                nc.vector.tensor_copy(t[:], raw[:])
                wo_ts.append(t)
            bo_row = wop.tile([P, D], f32)
            nc.scalar.dma_start(bo_row[:], bo.unsqueeze(0).to_broadcast((P, D)))
            ctx_ts = []
            for ec in range(DCH):
                t = wop.tile([P, S_LOC], f32r, tag=f"ctx{ec}", name=f"ctxt{ec}")
                nc.scalar.dma_start(t[:], ctxT_d[ec * P:(ec + 1) * P, :])
                ctx_ts.append(t)

            for sc in range(SCH):
                ps = oxp.tile([P, 2, NF], f32, tag="ox")
                for ec in range(DCH):
                    for fh in range(2):
                        nc.tensor.matmul(
                            ps[:, fh, :],
                            ctx_ts[ec][:, sc * P:(sc + 1) * P],
                            wo_ts[ec][:, fh * NF:(fh + 1) * NF],
                            start=(ec == 0), stop=(ec == DCH - 1),
                        )
                ob = osb.tile([P, D], f32, tag="ob")
                for fh in range(2):
                    nc.vector.tensor_add(
                        ob[:, fh * NF:(fh + 1) * NF], ps[:, fh, :],
                        bo_row[:, fh * NF:(fh + 1) * NF],
                    )
                nc.sync.dma_start(o_out[sc * P:(sc + 1) * P, :], ob[:])


def _get_program():
    if "nc" not in _CACHE:
        _CACHE["nc"] = _build_program()
    return _CACHE["nc"]


def kernel(query, key, value, Wq, bq, Wo, bo):
    global LAST_EXEC_NS
    query = np.ascontiguousarray(np.asarray(query, dtype=np.float32))
    key = np.ascontiguousarray(np.asarray(key, dtype=np.float32))
    value = np.ascontiguousarray(np.asarray(value, dtype=np.float32))
    Wq = np.ascontiguousarray(np.asarray(Wq, dtype=np.float32))
    bq = np.ascontiguousarray(np.asarray(bq, dtype=np.float32))
    Wo = np.ascontiguousarray(np.asarray(Wo, dtype=np.float32))
    bo = np.ascontiguousarray(np.asarray(bo, dtype=np.float32))

    B, S, Dm = query.shape
    assert (B, S, Dm) == (4, 2048, 1024), (B, S, Dm)

    nc = _get_program()
    wq_h = Wq.astype(ml_dtypes.bfloat16)
    wq_l = (Wq - wq_h.astype(np.float32)).astype(ml_dtypes.bfloat16)
    in_maps = []
    for c in range(8):
        b, sh = c // 2, c % 2
        in_maps.append({
            "q_in": np.ascontiguousarray(query[b, sh * S_LOC:(sh + 1) * S_LOC]),
            "k_in": np.ascontiguousarray(key[b, sh * (T // 2):(sh + 1) * (T // 2)]),
            "v_in": np.ascontiguousarray(value[b, sh * (T // 2):(sh + 1) * (T // 2)]),
            "Wq": Wq, "Wq_h": wq_h, "Wq_l": wq_l,
            "bq": bq, "Wo": Wo, "bo": bo,
        })

    res = run_bass_kernel_spmd(nc, in_maps, core_ids=list(range(8)))
    LAST_EXEC_NS = res.exec_time_ns

    out = np.empty((B, S, Dm), dtype=np.float32)
    weights = np.empty((B, S, T), dtype=np.float32)
    for c in range(8):
        b, sh = c // 2, c % 2
        out[b, sh * S_LOC:(sh + 1) * S_LOC] = res.results[c]["o_out"]
        weights[b, sh * S_LOC:(sh + 1) * S_LOC] = res.results[c]["w_out"]
    return out, weights


# revision 17
# speedup vs baseline: 1.0247x; 1.0103x over previous
"""Trainium2 Bass kernel for nn_Attention_49091476194121.

Single-head attention with a shared Q projection applied to q, k and v,
softmax (no scaling), then an output projection. Returns (out, weights)
exactly like the reference.

Sharding: 8 cores = 4 batches x 2 query-row halves. Each core handles one
(batch, s-half): it computes the full K/V projections for its batch
(duplicated across the pair) and its 1024 query rows end to end.

Precision: the q/k projections and the score matmul run in true fp32
(softmax exponentiates scores ~ +-130, so score errors are amplified);
the v projection, context matmul and output projection run in float32r
(TF32-class, ~1.5e-4 rel err, 4x faster on the PE).
"""

import os
import sys

sys.path.insert(0, "/opt/trn_rl_repo")

import ml_dtypes
import numpy as np

import concourse.bass as bass
import concourse.tile as tile
from concourse import bacc, mybir
from concourse.bass_utils import run_bass_kernel_spmd
from concourse.masks import make_identity

P = 128
D = 1024          # d_model
S_LOC = 1024      # query rows per core
T = 2048          # key/value rows per batch
DCH = D // P      # 8 chunks of the contraction/feature dims
SCH = S_LOC // P  # 8 query-row chunks
TCH = T // P      # 16 key-row chunks
NF = 512          # matmul free-dim tile

f32 = mybir.dt.float32
f32r = mybir.dt.float32r
bf16 = mybir.dt.bfloat16

# Set by kernel() when BASS_TRACE is enabled (see test.py).
LAST_EXEC_NS = None

_CACHE = {}


def _build_program():
    nc = bacc.Bacc("TRN2", target_bir_lowering=False, debug=False, num_devices=8)

    q_in = nc.dram_tensor("q_in", [S_LOC, D], f32, kind="ExternalInput").ap()
    k_in = nc.dram_tensor("k_in", [T // 2, D], f32, kind="ExternalInput").ap()
    v_in = nc.dram_tensor("v_in", [T // 2, D], f32, kind="ExternalInput").ap()
    Wq_h = nc.dram_tensor("Wq_h", [D, D], bf16, kind="ExternalInput").ap()
    Wq_l = nc.dram_tensor("Wq_l", [D, D], bf16, kind="ExternalInput").ap()
    Wq = nc.dram_tensor("Wq", [D, D], f32, kind="ExternalInput").ap()
    bq = nc.dram_tensor("bq", [D], f32, kind="ExternalInput").ap()
    Wo = nc.dram_tensor("Wo", [D, D], f32, kind="ExternalInput").ap()
    bo = nc.dram_tensor("bo", [D], f32, kind="ExternalInput").ap()

    w_out = nc.dram_tensor("w_out", [S_LOC, T], f32, kind="ExternalOutput").ap()
    o_out = nc.dram_tensor("o_out", [S_LOC, D], f32, kind="ExternalOutput").ap()

    # DRAM spills
    qpT_dh = nc.dram_tensor("qpT_dh", [D, S_LOC], bf16).ap()     # [e, s] hi
    qpT_dl = nc.dram_tensor("qpT_dl", [D, S_LOC], bf16).ap()     # [e, s] lo
    kloc_h = nc.dram_tensor("kloc_h", [D, T // 2], bf16).ap()    # local kpT half
    kloc_l = nc.dram_tensor("kloc_l", [D, T // 2], bf16).ap()
    kgat_h = nc.dram_tensor("kgat_h", [2 * D, T // 2], bf16).ap()
    kgat_l = nc.dram_tensor("kgat_l", [2 * D, T // 2], bf16).ap()
    vploc = nc.dram_tensor("vploc", [T // 2, D], f32r).ap()
    vp_d = nc.dram_tensor("vp_d", [T, D], f32r).ap()             # [t, e] gathered
    ctxT_d = nc.dram_tensor("ctxT_d", [D, S_LOC], f32r).ap()     # [e, s]
    wT_d = nc.dram_tensor("wT_d", [TCH, P, S_LOC], f32r).ap()    # [tc][t_in, s]

    with tile.TileContext(nc, pool_alloc_mode="queue") as tc:
        _emit(tc, nc, q_in, k_in, v_in, Wq_h, Wq_l, Wq, bq, Wo, bo,
              w_out, o_out, qpT_dh, qpT_dl, vp_d, wT_d,
              kloc_h, kloc_l, kgat_h, kgat_l, vploc, ctxT_d)

    nc.compile()
    return nc


def _transpose_rows_hl(nc, trpool, ident, row_ap, dst_h, dst_l, dst_col0, nch, scr):
    """PE-transpose + bf16 hi/lo split into dst_h/dst_l."""
    for dh in range((nch + 3) // 4):
        jmax = min(4, nch - dh * 4)
        tr = trpool.tile([P, 4, P], f32, tag="tr")
        for j in range(jmax):
            dch = dh * 4 + j
            nc.tensor.transpose(tr[:, j, :], row_ap[:, dch * P:(dch + 1) * P], ident[:])
        _split_hi_lo(
            nc, tr[:, :jmax, :],
            dst_h[:, dh * 4:dh * 4 + jmax, dst_col0:dst_col0 + P],
            dst_l[:, dh * 4:dh * 4 + jmax, dst_col0:dst_col0 + P],
            scr, [P, 4, P])


def _transpose_rows(nc, trpool, ident, row_ap, dst_st, dst_col0, nch):
    """PE-transpose row_ap ([128, nch*128], partition=rows) into dst_st
    ([128, nch, >=dst_col0+128], partition=cols), at free offset dst_col0."""
    for dh in range((nch + 3) // 4):
        jmax = min(4, nch - dh * 4)
        tr = trpool.tile([P, 4, P], f32, tag="tr")
        for j in range(jmax):
            dch = dh * 4 + j
            nc.tensor.transpose(tr[:, j, :], row_ap[:, dch * P:(dch + 1) * P], ident[:])
        nc.vector.tensor_copy(
            dst_st[:, dh * 4:dh * 4 + jmax, dst_col0:dst_col0 + P],
            tr[:, :jmax, :],
        )


def _split_hi_lo(nc, src_ap, hi_ap, lo_ap, scratch_pool, shape):
    """hi = bf16(src); lo = bf16(src - hi). src is f32 (SBUF or PSUM)."""
    nc.vector.tensor_copy(hi_ap, src_ap)
    h32 = scratch_pool.tile(shape, f32, tag="h32", name="h32")
    nc.vector.tensor_copy(h32[:], hi_ap)
    nc.vector.tensor_sub(lo_ap, src_ap, h32[:])


def _emit(tc, nc, q_in, k_in, v_in, Wq_hd, Wq_ld, Wq, bq, Wo, bo,
          w_out, o_out, qpT_dh, qpT_dl, vp_d, wT_d,
          kloc_h, kloc_l, kgat_h, kgat_l, vploc, ctxT_d):
    GROUPS = [[0, 1], [2, 3], [4, 5], [6, 7]]
    TQL = (T // 2) // NF   # 2 free-dim groups in the local t-half
    TCL = TCH // 2         # 8 row chunks in the local t-half

    ctx0 = tc.tile_pool(name="outer", bufs=1)
    with ctx0 as outer:
        ident = outer.tile([P, P], f32)
        make_identity(nc, ident[:])

        trp_cm = tc.tile_pool(name="trp", bufs=3, space="PSUM")
        with trp_cm as trpool:
            with (
                tc.tile_pool(name="wqbias", bufs=1) as wqp,
                tc.tile_pool(name="rows", bufs=2) as rows,
            ):
                bq_cols = wqp.tile([P, DCH], f32)    # bq_cols[p, ec] = bq[ec*128+p]
                nc.scalar.dma_start(bq_cols[:], bq.rearrange("(ec p) -> p ec", p=P))
                bq_row = wqp.tile([P, D], f32)       # bq broadcast to all partitions
                nc.scalar.dma_start(bq_row[:], bq.unsqueeze(0).to_broadcast((P, D)))
                wq_h = wqp.tile([P, DCH, D], bf16)   # Wq hi [d-part, dch, e]
                wq_l = wqp.tile([P, DCH, D], bf16)   # Wq lo
                for dch in range(DCH):
                    nc.scalar.dma_start(wq_h[:, dch, :], Wq_hd[dch * P:(dch + 1) * P, :])
                    nc.scalar.dma_start(wq_l[:, dch, :], Wq_ld[dch * P:(dch + 1) * P, :])

                # ---- K (local t-half): kpT_loc -> kloc_h/l -> AllGather ----
                with (
                    tc.tile_pool(name="scr", bufs=2) as scr,
                    tc.tile_pool(name="xst", bufs=2) as xst,
                    tc.tile_pool(name="spill", bufs=3) as spill,
                    tc.tile_pool(name="pp", bufs=2, space="PSUM") as pp,
                ):
                    for tq in range(TQL):
                        k_sh = xst.tile([P, DCH, NF], bf16, tag="xsth")
                        k_sl = xst.tile([P, DCH, NF], bf16, tag="xstl")
                        for ts in range(4):
                            r = rows.tile([P, D], f32, tag="row")
                            nc.sync.dma_start(r[:], k_in[(tq * 4 + ts) * P:(tq * 4 + ts + 1) * P, :])
                            _transpose_rows_hl(nc, trpool, ident, r[:], k_sh, k_sl, ts * P, DCH, scr)
                        for ecg in range(4):
                            ps = pp.tile([P, 2, NF], f32, tag="proj")
                            for d in range(DCH):
                                for e2 in range(2):
                                    ec = ecg * 2 + e2
                                    first = (d == 0)
                                    last = (d == DCH - 1)
                                    nc.tensor.matmul(
                                        ps[:, e2, :], wq_h[:, d, ec * P:(ec + 1) * P], k_sh[:, d, :],
                                        start=first, stop=False)
                                    nc.tensor.matmul(
                                        ps[:, e2, :], wq_h[:, d, ec * P:(ec + 1) * P], k_sl[:, d, :],
                                        start=False, stop=False)
                                    nc.tensor.matmul(
                                        ps[:, e2, :], wq_l[:, d, ec * P:(ec + 1) * P], k_sh[:, d, :],
                                        start=False, stop=last)
                            for e2 in range(2):
                                ec = ecg * 2 + e2
                                kf = scr.tile([P, NF], f32, tag="qf", name="kf")
                                nc.vector.tensor_scalar_add(kf[:], ps[:, e2, :], bq_cols[:, ec:ec + 1])
                                kh = spill.tile([P, NF], bf16, tag="qsph", name="kh")
                                kl = spill.tile([P, NF], bf16, tag="qspl", name="kl")
                                _split_hi_lo(nc, kf[:], kh[:], kl[:], scr, [P, NF])
                                nc.sync.dma_start(
                                    kloc_h[ec * P:(ec + 1) * P, tq * NF:(tq + 1) * NF], kh[:])
                                nc.sync.dma_start(
                                    kloc_l[ec * P:(ec + 1) * P, tq * NF:(tq + 1) * NF], kl[:])
                nc.gpsimd.collective_compute(
                    "AllGather", mybir.AluOpType.bypass, replica_groups=GROUPS,
                    ins=[kloc_h], outs=[kgat_h])
                nc.gpsimd.collective_compute(
                    "AllGather", mybir.AluOpType.bypass, replica_groups=GROUPS,
                    ins=[kloc_l], outs=[kgat_l])

                # ---- V (local t-half): vp_loc -> AllGather -> vp_d  (f32r) ----
                with (
                    tc.tile_pool(name="wqr", bufs=1) as wqr,
                    tc.tile_pool(name="vstp", bufs=2) as vstp,
                    tc.tile_pool(name="vspill", bufs=2) as vspill,
                    tc.tile_pool(name="ppv", bufs=2, space="PSUM") as ppv,
                ):
                    wq_r = wqr.tile([P, DCH, D], f32r)
                    for dch in range(DCH):
                        nc.vector.tensor_add(wq_r[:, dch, :], wq_h[:, dch, :], wq_l[:, dch, :])

                    for tch in range(TCL):
                        r = rows.tile([P, D], f32, tag="row")
                        nc.sync.dma_start(r[:], v_in[tch * P:(tch + 1) * P, :])
                        v_st = vstp.tile([P, DCH, P], f32r, tag="vst")
                        _transpose_rows(nc, trpool, ident, r[:], v_st, 0, DCH)
                        ps = ppv.tile([P, 2, NF], f32, tag="vproj")
                        for d in range(DCH):
                            for eh in range(2):
                                nc.tensor.matmul(
                                    ps[:, eh, :],
                                    v_st[:, d, :],
                                    wq_r[:, d, eh * NF:(eh + 1) * NF],
                                    start=(d == 0), stop=(d == DCH - 1),
                                )
                        vsb = vspill.tile([P, D], f32r, tag="vsp")
                        for eh in range(2):
                            nc.vector.tensor_add(
                                vsb[:, eh * NF:(eh + 1) * NF], ps[:, eh, :],
                                bq_row[:, eh * NF:(eh + 1) * NF],
                            )
                        nc.sync.dma_start(vploc[tch * P:(tch + 1) * P, :], vsb[:])
                nc.gpsimd.collective_compute(
                    "AllGather", mybir.AluOpType.bypass, replica_groups=GROUPS,
                    ins=[vploc], outs=[vp_d])

                # ---- Q: qpT[e, s] = (q_in @ Wq + bq)^T -> spill hi/lo ----
                with (
                    tc.tile_pool(name="scr3", bufs=2) as scr,
                    tc.tile_pool(name="xst3", bufs=2) as xst,
                    tc.tile_pool(name="spill3", bufs=3) as spill,
                    tc.tile_pool(name="pp3", bufs=2, space="PSUM") as pp,
                ):
                    for sh in range(2):
                        q_sh = xst.tile([P, DCH, NF], bf16, tag="xsth")
                        q_sl = xst.tile([P, DCH, NF], bf16, tag="xstl")
                        for ss in range(4):
                            r = rows.tile([P, D], f32, tag="row")
                            nc.scalar.dma_start(r[:], q_in[(sh * 4 + ss) * P:(sh * 4 + ss + 1) * P, :])
                            _transpose_rows_hl(nc, trpool, ident, r[:], q_sh, q_sl, ss * P, DCH, scr)
                        for ecg in range(4):
                            ps = pp.tile([P, 2, NF], f32, tag="proj")
                            for d in range(DCH):
                                for e2 in range(2):
                                    ec = ecg * 2 + e2
                                    first = (d == 0)
                                    last = (d == DCH - 1)
                                    nc.tensor.matmul(
                                        ps[:, e2, :], wq_h[:, d, ec * P:(ec + 1) * P], q_sh[:, d, :],
                                        start=first, stop=False)
                                    nc.tensor.matmul(
                                        ps[:, e2, :], wq_h[:, d, ec * P:(ec + 1) * P], q_sl[:, d, :],
                                        start=False, stop=False)
                                    nc.tensor.matmul(
                                        ps[:, e2, :], wq_l[:, d, ec * P:(ec + 1) * P], q_sh[:, d, :],
                                        start=False, stop=last)
                            for e2 in range(2):
                                ec = ecg * 2 + e2
                                qf = scr.tile([P, NF], f32, tag="qf", name="qf")
                                nc.vector.tensor_scalar_add(qf[:], ps[:, e2, :], bq_cols[:, ec:ec + 1])
                                sbh = spill.tile([P, NF], bf16, tag="qsph", name="sbh")
                                sbl = spill.tile([P, NF], bf16, tag="qspl", name="sbl")
                                _split_hi_lo(nc, qf[:], sbh[:], sbl[:], scr, [P, NF])
                                nc.sync.dma_start(
                                    qpT_dh[ec * P:(ec + 1) * P, sh * NF:(sh + 1) * NF], sbh[:])
                                nc.sync.dma_start(
                                    qpT_dl[ec * P:(ec + 1) * P, sh * NF:(sh + 1) * NF], sbl[:])

            # ---- S: scores/softmax/wT + fused-resident CTX ----
            with (
                tc.tile_pool(name="kres", bufs=1) as res,
                tc.tile_pool(name="qpt", bufs=1) as qptp,
                tc.tile_pool(name="vpt", bufs=1) as vptp,
            ):
                kpT_hs, kpT_ls = [], []
                for ec in range(DCH):
                    th = res.tile([P, T], bf16, tag=f"kpth{ec}", name=f"kpTh{ec}")
                    tl = res.tile([P, T], bf16, tag=f"kptl{ec}", name=f"kpTl{ec}")
                    for rk in range(2):
                        nc.scalar.dma_start(
                            th[:, rk * (T // 2):(rk + 1) * (T // 2)],
                            kgat_h[rk * D + ec * P:rk * D + (ec + 1) * P, :])
                        nc.scalar.dma_start(
                            tl[:, rk * (T // 2):(rk + 1) * (T // 2)],
                            kgat_l[rk * D + ec * P:rk * D + (ec + 1) * P, :])
                    kpT_hs.append(th)
                    kpT_ls.append(tl)
                qpT_hs, qpT_ls = [], []
                for ec in range(DCH):
                    th = qptp.tile([P, S_LOC], bf16, tag=f"qpth{ec}", name=f"qpTh{ec}")
                    nc.scalar.dma_start(th[:], qpT_dh[ec * P:(ec + 1) * P, :])
                    tl = qptp.tile([P, S_LOC], bf16, tag=f"qptl{ec}", name=f"qpTl{ec}")
                    nc.scalar.dma_start(tl[:], qpT_dl[ec * P:(ec + 1) * P, :])
                    qpT_hs.append(th)
                    qpT_ls.append(tl)
                vp_ts = []
                for tch in range(TCH):
                    t = vptp.tile([P, D], f32r, tag=f"vp{tch}", name=f"vpt{tch}")
                    nc.scalar.dma_start(t[:], vp_d[tch * P:(tch + 1) * P, :])
                    vp_ts.append(t)

                with (
                    tc.tile_pool(name="sm", bufs=2) as sm,
                    tc.tile_pool(name="scp", bufs=5, space="PSUM") as scp,
                ):
                    for sc in range(SCH):
                        sct = [scp.tile([P, NF], f32, tag="sc", name=f"sct{i}") for i in range(4)]
                        for ec in range(DCH):
                            first = (ec == 0)
                            last = (ec == DCH - 1)
                            for tq in range(4):
                                nc.tensor.matmul(
                                    sct[tq][:],
                                    qpT_hs[ec][:, sc * P:(sc + 1) * P],
                                    kpT_hs[ec][:, tq * NF:(tq + 1) * NF],
                                    start=first, stop=False)
                            for tq in range(4):
                                nc.tensor.matmul(
                                    sct[tq][:],
                                    qpT_hs[ec][:, sc * P:(sc + 1) * P],
                                    kpT_ls[ec][:, tq * NF:(tq + 1) * NF],
                                    start=False, stop=False)
                            for tq in range(4):
                                nc.tensor.matmul(
                                    sct[tq][:],
                                    qpT_ls[ec][:, sc * P:(sc + 1) * P],
                                    kpT_hs[ec][:, tq * NF:(tq + 1) * NF],
                                    start=False, stop=last)
                        sc_sb = sm.tile([P, T], f32, tag="scsb")
                        for tq in range(4):
                            nc.vector.tensor_copy(sc_sb[:, tq * NF:(tq + 1) * NF], sct[tq][:])
                        pmax = sm.tile([P, 4], f32, tag="pmax")
                        for tq in range(4):
                            nc.vector.tensor_reduce(
                                pmax[:, tq:tq + 1], sc_sb[:, tq * NF:(tq + 1) * NF],
                                axis=mybir.AxisListType.X, op=mybir.AluOpType.max,
                            )
                        negmax = sm.tile([P, 1], f32, tag="negmax")
                        nc.vector.tensor_reduce(
                            negmax[:], pmax[:], axis=mybir.AxisListType.X,
                            op=mybir.AluOpType.max, negate=True,
                        )
                        w_sb = sm.tile([P, T], f32, tag="wsb", bufs=1)
                        sums = sm.tile([P, 4], f32, tag="sums")
                        for tq in range(4):
                            nc.scalar.activation(
                                w_sb[:, tq * NF:(tq + 1) * NF], sc_sb[:, tq * NF:(tq + 1) * NF],
                                mybir.ActivationFunctionType.Exp,
                                bias=negmax[:], scale=1.0,
                                accum_out=sums[:, tq:tq + 1],
                            )
                        stot = sm.tile([P, 1], f32, tag="stot")
                        nc.vector.tensor_reduce(
                            stot[:], sums[:], axis=mybir.AxisListType.X, op=mybir.AluOpType.add,
                        )
                        recip = sm.tile([P, 1], f32, tag="recip")
                        nc.vector.reciprocal(recip[:], stot[:])
                        for tq in range(4):
                            nc.vector.tensor_scalar_mul(
                                w_sb[:, tq * NF:(tq + 1) * NF],
                                w_sb[:, tq * NF:(tq + 1) * NF], recip[:],
                            )
                        nc.sync.dma_start(w_out[sc * P:(sc + 1) * P, :], w_sb[:])

                        wT_sb = sm.tile([P, TCH, P], f32r, tag="wtsb")
                        for th in range(4):
                            tr = trpool.tile([P, 4, P], f32, tag="tr")
                            for j in range(4):
                                tch = th * 4 + j
                                nc.tensor.transpose(
                                    tr[:, j, :], w_sb[:, tch * P:(tch + 1) * P], ident[:]
                                )
                            nc.vector.tensor_copy(wT_sb[:, th * 4:(th + 1) * 4, :], tr[:])
                        nc.sync.dma_start(
                            wT_d.rearrange("tc p s -> p tc s")[:, :, sc * P:(sc + 1) * P],
                            wT_sb[:],
                        )

                # ---- context: ctxT[e, s] = vp^T @ w^T  (f32r) -> ctxT_d ----
                with (
                    tc.tile_pool(name="wtin", bufs=4) as wtin,
                    tc.tile_pool(name="cxs", bufs=2) as cxs,
                    tc.tile_pool(name="cxp", bufs=1, space="PSUM") as cxp,
                ):
                    for sh in range(2):
                        for g in range(2):
                            ps = cxp.tile([P, 4, NF], f32, tag="cx", name=f"cx{sh}{g}")
                            for tch in range(TCH):
                                wt = wtin.tile([P, NF], f32r, tag="wt")
                                nc.scalar.dma_start(
                                    wt[:], wT_d[tch, :, sh * NF:(sh + 1) * NF]
                                )
                                for e4 in range(4):
                                    ec = g * 4 + e4
                                    nc.tensor.matmul(
                                        ps[:, e4, :],
                                        vp_ts[tch][:, ec * P:(ec + 1) * P],
                                        wt[:],
                                        start=(tch == 0), stop=(tch == TCH - 1),
                                    )
                            st = cxs.tile([P, 4, NF], f32r, tag="cxs", name=f"cxs{sh}{g}")
                            nc.vector.tensor_copy(st[:], ps[:])
                            nc.sync.dma_start(
                                ctxT_d.rearrange("(a p) s -> p a s", p=P)[
                                    :, g * 4:(g + 1) * 4, sh * NF:(sh + 1) * NF],
                                st[:],
                            )

        # ---- output projection: out[s, f] = ctxT^T @ Wo + bo  (f32r) ----
        with (
            tc.tile_pool(name="wo", bufs=1) as wop,
            tc.tile_pool(name="osb", bufs=2) as osb,
            tc.tile_pool(name="oxp", bufs=2, space="PSUM") as oxp,
        ):
            wo_ts = []
            for ec in range(DCH):
                raw = wop.tile([P, D], f32, tag="woraw", name=f"woraw{ec}", bufs=3)
                nc.scalar.dma_start(raw[:], Wo[ec * P:(ec + 1) * P, :])
                t = wop.tile([P, D], f32r, tag=f"wo{ec}", name=f"wo{ec}")
                nc.vector.tensor_copy(t[:], raw[:])
                wo_ts.append(t)
            bo_row = wop.tile([P, D], f32)
            nc.scalar.dma_start(bo_row[:], bo.unsqueeze(0).to_broadcast((P, D)))
            ctx_ts = []
            for ec in range(DCH):
                t = wop.tile([P, S_LOC], f32r, tag=f"ctx{ec}", name=f"ctxt{ec}")
                nc.scalar.dma_start(t[:], ctxT_d[ec * P:(ec + 1) * P, :])
                ctx_ts.append(t)

            for sc in range(SCH):
                ps = oxp.tile([P, 2, NF], f32, tag="ox")
                for ec in range(DCH):
                    for fh in range(2):
                        nc.tensor.matmul(
                            ps[:, fh, :],
                            ctx_ts[ec][:, sc * P:(sc + 1) * P],
                            wo_ts[ec][:, fh * NF:(fh + 1) * NF],
                            start=(ec == 0), stop=(ec == DCH - 1),
                        )
                ob = osb.tile([P, D], f32, tag="ob")
                for fh in range(2):
                    nc.vector.tensor_add(
                        ob[:, fh * NF:(fh + 1) * NF], ps[:, fh, :],
                        bo_row[:, fh * NF:(fh + 1) * NF],
                    )
                nc.sync.dma_start(o_out[sc * P:(sc + 1) * P, :], ob[:])


def _get_program():
    if "nc" not in _CACHE:
        _CACHE["nc"] = _build_program()
    return _CACHE["nc"]


def kernel(query, key, value, Wq, bq, Wo, bo):
    global LAST_EXEC_NS
    query = np.ascontiguousarray(np.asarray(query, dtype=np.float32))
    key = np.ascontiguousarray(np.asarray(key, dtype=np.float32))
    value = np.ascontiguousarray(np.asarray(value, dtype=np.float32))
    Wq = np.ascontiguousarray(np.asarray(Wq, dtype=np.float32))
    bq = np.ascontiguousarray(np.asarray(bq, dtype=np.float32))
    Wo = np.ascontiguousarray(np.asarray(Wo, dtype=np.float32))
    bo = np.ascontiguousarray(np.asarray(bo, dtype=np.float32))

    B, S, Dm = query.shape
    assert (B, S, Dm) == (4, 2048, 1024), (B, S, Dm)

    nc = _get_program()
    wq_h = Wq.astype(ml_dtypes.bfloat16)
    wq_l = (Wq - wq_h.astype(np.float32)).astype(ml_dtypes.bfloat16)
    in_maps = []
    for c in range(8):
        b, sh = c // 2, c % 2
        in_maps.append({
            "q_in": np.ascontiguousarray(query[b, sh * S_LOC:(sh + 1) * S_LOC]),
            "k_in": np.ascontiguousarray(key[b, sh * (T // 2):(sh + 1) * (T // 2)]),
            "v_in": np.ascontiguousarray(value[b, sh * (T // 2):(sh + 1) * (T // 2)]),
            "Wq": Wq, "Wq_h": wq_h, "Wq_l": wq_l,
            "bq": bq, "Wo": Wo, "bo": bo,
        })

    res = run_bass_kernel_spmd(nc, in_maps, core_ids=list(range(8)))
    LAST_EXEC_NS = res.exec_time_ns

    out = np.empty((B, S, Dm), dtype=np.float32)
    weights = np.empty((B, S, T), dtype=np.float32)
    for c in range(8):
        b, sh = c // 2, c % 2
        out[b, sh * S_LOC:(sh + 1) * S_LOC] = res.results[c]["o_out"]
        weights[b, sh * S_LOC:(sh + 1) * S_LOC] = res.results[c]["w_out"]
    return out, weights
